# revision 13
# baseline (speedup 1.0000x reference)
"""ColumnParallelLinear + paged LoRA (SGMV) on 8 trn2 NeuronCores.

Math (per reference):
    out = x @ W^T + bias;  out[t] += x[t] @ A[l(t)] @ B[l(t)]
where l(t) is the adapter of token t's contiguous segment (from `indices`).

Sharding: column-parallel over the output dim.  Core c owns O/8 = 512
output columns: W shard, bias shard, B shard; x, A and the segment map are
replicated.

Precision: the whole compute path runs in bf16 operands with fp32 PSUM
accumulation (measured end-to-end relative error ~2e-3 against a 2e-2
budget).  bf16 halves the dominant DMA traffic (replicated x).

Device layout: everything is computed transposed (out^T [O_s, T]) so the
contraction dim H lands on SBUF partitions for both matmul operands.  x
and W are pre-packed on the host into [128, PK*512] blocks of PK=4
k-tiles so every DMA moves 4 KB per partition line.

LoRA handling (all matmuls full 128-wide contraction; 16-partition
matmuls measure ~2x slower per row on TRN2):
  *  u_all = x @ A_all  for ALL adapters at once ([L*R = 128] rank rows),
     masked per token down to the active adapter's 16 rows (mask built on
     the host from `indices`), then folded into the main GEMM as one
     extra contraction step: out += B_all^T @ u_masked.
  *  u_all is k-sharded across the cores (each core contracts its own H/8
     slice using x blocks it streams anyway; host-side k-rotation of the
     x/W blocks keeps the SPMD program identical on all cores) and ONE
     AllReduce assembles it.  The ncfw collective has ~50-90 us latency,
     so the first E tiles instead compute u_all locally (full k) as a 5th
     stationary column in their own j-loop; only tiles >= E use the
     collective result, by which time it has long completed.

The program is independent of `indices` (the segment map only enters
through the host-built mask tensor), so one compiled NEFF serves any
input.
"""

import numpy as np
import ml_dtypes

import concourse.bass as bass
import concourse.mybir as mybir
import concourse.tile as tile
from concourse.tile import TileContext
from concourse.vector_clock import ScopedClock

N_CORES = 8
T, H, O, R, L = 4096, 4096, 4096, 16, 8
O_S = O // N_CORES

F32 = mybir.dt.float32
F32R = mybir.dt.float32r
BF16 = mybir.dt.bfloat16
NP_BF16 = np.dtype(ml_dtypes.bfloat16)

PK = 4        # k-tiles packed per DMA block (4 KB partition lines)
E_LOCAL = 3   # tiles computing u locally (covers AllReduce latency)

_drain_patched = False


def _patch_drain_waits():
    """walrus in this image rejects >1 sync-wait on the Tile exit Drain;
    spill the extra waits onto SP nops (semantically identical: SP
    executes them in order before the all-engine barrier)."""
    global _drain_patched
    if _drain_patched:
        return
    _drain_patched = True

    def _drain_and_barrier(self, tick_clock, wait_clock):
        drain_inst = self.nc.sync.drain()
        wait_clock.add_sem_waits(
            drain_inst.ins, ScopedClock({None: tick_clock.global_clock})
        )
        si = drain_inst.ins.sync_info
        if si is not None and si.on_wait and len(si.on_wait) > 1:
            waits = list(si.on_wait)
            si.on_wait = waits[:1]
            for w in waits[1:]:
                nop = self.nc.sync.nop()
                nop.ins.sync_info = mybir.SyncInfo(on_wait=[w], on_update=[])
        self.nc.all_engine_barrier()
        assert self.sems is not None
        popped = self.nc._tile_sem_poison_stack.pop()
        assert popped is self._sem_poison
        self.nc.clear_and_free_semaphores(list(self.sems.allocated().values()))
        self.nc.all_engine_barrier()

    TileContext._drain_and_barrier = _drain_and_barrier


def _split_instruction_waits(nc, chain_sem, max_waits=1, verbose=False):
    """walrus in this image encodes at most one sync-wait per instruction.

    Engine instructions execute in stream order, so extra waits can be
    peeled onto NoOps inserted immediately before the instruction.  For
    DMA transfers (whose single wait may be evaluated by the DGE queue
    rather than the issuing sequencer) all original waits are funnelled
    through SP NoOps that bump a dedicated chain semaphore; the DMA then
    waits for the chain count, which is equivalent to the conjunction of
    its original waits."""
    fn = nc.m.functions[0]
    stats = {}
    chain_used = False
    chain_count = 0
    for blk in fn.blocks:
        out = []
        changed = False
        for inst in blk.instructions:
            si = getattr(inst, "sync_info", None)
            if si is not None and si.on_wait and len(si.on_wait) > max_waits:
                stats[inst.opcode] = stats.get(inst.opcode, 0) + 1
                waits = list(si.on_wait)
                changed = True
                if "DMA" in inst.opcode:
                    chain_used = True
                    chain_count += 1
                    for idx, w in enumerate(waits):
                        nop = mybir.InstNoOp(
                            name=nc.get_next_instruction_name(),
                            engine=mybir.EngineType.SP,
                        )
                        upd = []
                        if idx == len(waits) - 1:
                            upd = [
                                mybir.SyncUpdate(
                                    sync_type="semaphore",
                                    id=chain_sem.num,
                                    update_mode="sem-inc",
                                    ant_name=chain_sem.name,
                                    update_value=1,
                                )
                            ]
                        nop.sync_info = mybir.SyncInfo(on_wait=[w], on_update=upd)
                        nc.register_instruction(nop)
                        out.append(nop)
                    si.on_wait = [
                        mybir.SyncWait(
                            sync_type="semaphore",
                            id=chain_sem.num,
                            wait_mode="sem-ge-imm",
                            ant_name=chain_sem.name,
                            wait_value=chain_count,
                        )
                    ]
                else:
                    for w in waits[:-max_waits]:
                        nop = mybir.InstNoOp(
                            name=nc.get_next_instruction_name(), engine=inst.engine
                        )
                        nop.sync_info = mybir.SyncInfo(on_wait=[w], on_update=[])
                        nc.register_instruction(nop)
                        out.append(nop)
                    si.on_wait = waits[-max_waits:]
            out.append(inst)
        if changed:
            blk.instructions = out
    if chain_used:
        # Reset the chain sem after the tail barrier so NEFF re-execution
        # starts from zero.
        nc.sync.sem_clear(chain_sem)
    if verbose and stats:
        print("split multi-wait instructions:", stats)
    return stats


def _install_ntff_shim():
    """Provide antenv.axon_hooks (absent in this image) so
    run_bass_kernel_spmd(trace=True) can capture NTFF profiles through
    the axon sidechannel, mirroring trn_boot's ctypes hook."""
    try:
        import antenv.axon_hooks  # noqa: F401
        return
    except ImportError:
        pass
    import contextlib
    import ctypes
    import sys
    import types

    import antenv

    mod = types.ModuleType("antenv.axon_hooks")
    holder = {}
    mod.set_axon_ntff_profile_hook = lambda h: holder.__setitem__("h", h)
    mod.get_axon_ntff_profile_hook = lambda: holder.get("h")
    sys.modules["antenv.axon_hooks"] = mod
    antenv.axon_hooks = mod

    so_path = "/opt/axon/libaxon_pjrt.so"
    lib = ctypes.CDLL(so_path)
    if not hasattr(lib, "axon_start_nrt_profile"):
        return
    lib.axon_start_nrt_profile.argtypes = [
        ctypes.POINTER(ctypes.c_int64),
        ctypes.c_size_t,
    ]
    lib.axon_start_nrt_profile.restype = ctypes.c_int64
    lib.axon_stop_nrt_profile.argtypes = [ctypes.c_char_p]
    lib.axon_stop_nrt_profile.restype = ctypes.c_int64

    @contextlib.contextmanager
    def _hook(output_dir, device_ids):
        import jax

        jax.devices()
        if device_ids:
            ids = (ctypes.c_int64 * len(device_ids))(*device_ids)
            rc = lib.axon_start_nrt_profile(ids, len(device_ids))
        else:
            rc = lib.axon_start_nrt_profile(None, 0)
        if rc != 0:
            raise RuntimeError(f"axon_start_nrt_profile rc={rc}")
        try:
            yield
        finally:
            n = lib.axon_stop_nrt_profile(str(output_dir).encode())
            print(f"ntff profile: {n} file(s) written to {output_dir}")

    mod.set_axon_ntff_profile_hook(_hook)


def runs_from_indices(indices: np.ndarray, n_tokens: int) -> tuple:
    """Expand `indices` into maximal contiguous token runs with a fixed
    adapter, mirroring the reference searchsorted semantics exactly
    (including the negative-index wrap for tokens before starts[0])."""
    starts = np.asarray(indices[:-1, 0], dtype=np.int64)
    seg_lora = np.asarray(indices[:-1, 1], dtype=np.int64)
    tok = np.arange(n_tokens, dtype=np.int64)
    seg = np.searchsorted(starts, tok, side="right") - 1
    tok_lora = seg_lora[seg]  # seg == -1 wraps to the last segment, like jnp
    change = np.flatnonzero(np.diff(tok_lora)) + 1
    run_starts = np.concatenate(([0], change))
    run_ends = np.concatenate((change, [n_tokens]))
    return tuple(
        (int(a), int(b), int(tok_lora[a])) for a, b in zip(run_starts, run_ends)
    )


def build_program(t=T, h=H, o_s=O_S, r=R, n_lora=L, n_shards=N_CORES,
                  e_local=E_LOCAL):
    """Emit the single-core Tile program (SPMD across the cores)."""
    _patch_drain_waits()
    assert t % 512 == 0 and h % 128 == 0 and o_s % 128 == 0
    kt = h // 128          # contraction tiles
    nt = t // 512          # token (moving) tiles
    mt = o_s // 128        # output-partition tiles
    ra = n_lora * r        # all-adapter rank width
    assert ra == 128
    assert kt % n_shards == 0
    ksh = kt // n_shards   # k-tiles per core's u shard
    assert ksh == PK, "prepass consumes exactly the first packed block"
    kb_cnt = kt // PK
    e_local = max(1, min(e_local, nt))

    nc = bass.Bass("TRN2", num_devices=n_shards)
    chain_sem = nc.alloc_semaphore("dma_wait_chain")
    xB_d = nc.dram_tensor("xB", [kb_cnt, nt, 128, PK * 512], BF16,
                          kind="ExternalInput")
    wB_d = nc.dram_tensor("wB", [kb_cnt, 128, PK * o_s], BF16,
                          kind="ExternalInput")
    aT_d = nc.dram_tensor("aT", [128, kt * ra // 128, 128], BF16,
                          kind="ExternalInput")
    ball_d = nc.dram_tensor("ball", [128, o_s], BF16, kind="ExternalInput")
    mask_d = nc.dram_tensor("umask", [nt, 128, 512], BF16, kind="ExternalInput")
    bias_d = nc.dram_tensor("bias_r", [128, mt], F32, kind="ExternalInput")
    out_d = nc.dram_tensor("outT", [o_s, t], F32, kind="ExternalOutput")

    aT_v = aT_d[:].rearrange("p a b -> p (a b)")

    with TileContext(nc) as tc:
        with (
            tc.tile_pool(name="resident", bufs=1) as res,
            tc.tile_pool(name="xs", bufs=8) as xs,
            tc.tile_pool(name="xpre", bufs=max(1, nt - e_local)) as xpre,
            tc.tile_pool(name="masks", bufs=nt) as maskp,
            tc.tile_pool(name="us", bufs=4) as us,
            tc.tile_pool(name="ufs", bufs=max(1, nt - e_local)) as ufs,
            tc.tile_pool(name="outs", bufs=6) as outs,
            tc.tile_pool(name="dramp", bufs=1, space="DRAM") as dramp,
            tc.tile_pool(name="psum_o", bufs=6, space="PSUM") as psum_o,
            tc.tile_pool(name="psum_u", bufs=2, space="PSUM") as psum_u,
        ):
            w_sb = res.tile([128, kt * o_s], BF16, tag="w", name="w_sb")
            a_sb = res.tile([128, kt * ra], BF16, tag="a", name="a_sb")
            ball_sb = res.tile([128, o_s], BF16, tag="ball", name="ball_sb")
            bias_sb = res.tile([128, mt], F32, tag="bias", name="bias_sb")

            up_t = dramp.tile([nt, 128, 512], BF16, tag="upart", name="up_t")
            uf_t = dramp.tile(
                [max(1, nt - e_local), 128, 512], BF16, tag="ufull",
                name="uf_t", addr_space="Shared",
            )

            xpre_tiles = {}
            mask_tiles = {}

            def load_w(kb):
                nc.sync.dma_start(
                    w_sb[:, kb * PK * o_s:(kb + 1) * PK * o_s], wB_d[kb]
                )

            # ---- warm-up.  Stream the prepass x blocks (they double as
            # the kb=0 blocks of tiles >= e_local) and this core's u-shard
            # partials, then issue the single AllReduce as early as
            # possible: ncfw collectives take ~50-90 us end to end, which
            # the first e_local (locally-computed) tiles hide.
            nc.sync.dma_start(a_sb[:], aT_v)
            for n in range(e_local, nt):
                xt = xpre.tile([128, PK * 512], BF16, tag="xp", name="xp")
                xpre_tiles[n] = xt
                nc.sync.dma_start(xt[:], xB_d[0, n])
            for kb in range(min(2, kb_cnt)):
                load_w(kb)
            for n in range(e_local, nt):
                pu = psum_u.tile([128, 512], F32, tag="pu", name="pu")
                xt = xpre_tiles[n]
                for jj in range(ksh):
                    nc.tensor.matmul(
                        pu[:],
                        a_sb[:, jj * ra:(jj + 1) * ra],
                        xt[:, jj * 512:(jj + 1) * 512],
                        start=(jj == 0),
                        stop=(jj == ksh - 1),
                    )
                up_sb = us.tile([128, 512], BF16, tag="up", name="up_sb")
                nc.vector.tensor_copy(up_sb[:], pu[:])
                nc.sync.dma_start(up_t[n], up_sb[:])
            if e_local < nt:
                nc.gpsimd.collective_compute(
                    "AllReduce",
                    mybir.AluOpType.add,
                    replica_groups=[list(range(n_shards))],
                    ins=[up_t[e_local:nt]],
                    outs=[uf_t[:]],
                )
            nc.sync.dma_start(ball_sb[:], ball_d[:])
            nc.sync.dma_start(bias_sb[:], bias_d[:])
            for kb in range(2, kb_cnt):
                load_w(kb)
            for n in range(nt):
                mk = maskp.tile([128, 512], BF16, tag="mk", name="mk")
                mask_tiles[n] = mk
                nc.sync.dma_start(mk[:], mask_d[n])

            uf_tiles = {}

            def main_tile(n):
                c0 = n * 512
                local = n < e_local
                ptiles = [
                    psum_o.tile([128, 512], F32, tag="po", name="po")
                    for _ in range(mt)
                ]
                if local:
                    pu = psum_u.tile([128, 512], F32, tag="pu", name="pu")
                for kb in range(kb_cnt):
                    if kb == 0 and n in xpre_tiles:
                        xt = xpre_tiles[n]
                    else:
                        xt = xs.tile([128, PK * 512], BF16, tag="x", name="xt")
                        nc.sync.dma_start(xt[:], xB_d[kb, n])
                    for q in range(PK):
                        j = kb * PK + q
                        mv = xt[:, q * 512:(q + 1) * 512]
                        for m in range(mt):
                            nc.tensor.matmul(
                                ptiles[m][:],
                                w_sb[:, j * o_s + m * 128:j * o_s + (m + 1) * 128],
                                mv,
                                start=(j == 0),
                                stop=False,
                            )
                        if local:
                            nc.tensor.matmul(
                                pu[:],
                                a_sb[:, j * ra:(j + 1) * ra],
                                mv,
                                start=(j == 0),
                                stop=(j == kt - 1),
                            )
                # masked u for this tile (bf16), then fold B_all in as one
                # extra contraction step per output block.
                um = us.tile([128, 512], BF16, tag="um", name="um")
                if local:
                    uraw = us.tile([128, 512], BF16, tag="ur", name="ur")
                    nc.vector.tensor_copy(uraw[:], pu[:])
                    nc.vector.tensor_tensor(
                        um[:], uraw[:], mask_tiles[n][:], mybir.AluOpType.mult
                    )
                else:
                    nc.vector.tensor_tensor(
                        um[:], uf_tiles[n][:], mask_tiles[n][:],
                        mybir.AluOpType.mult,
                    )
                for m in range(mt):
                    nc.tensor.matmul(
                        ptiles[m][:],
                        ball_sb[:, m * 128:(m + 1) * 128],
                        um[:],
                        start=False,
                        stop=True,
                    )
                for m in range(mt):
                    ot = outs.tile([128, 512], F32, tag="o", name="ot")
                    nc.vector.tensor_scalar_add(
                        ot[:], ptiles[m][:], bias_sb[:, m:m + 1]
                    )
                    nc.sync.dma_start(out_d[m * 128:(m + 1) * 128, c0:c0 + 512],
                                      ot[:])
                if n == e_local - 1:
                    # The AllReduce has had ~3 tiles of runway; pull its
                    # results into SBUF in one batch (these DMAs wait on
                    # the collective, so they are kept off the x-stream's
                    # critical path until now).
                    for nn in range(e_local, nt):
                        uf = ufs.tile([128, 512], BF16, tag="uf", name="uf")
                        uf_tiles[nn] = uf
                        nc.sync.dma_start(uf[:], uf_t[nn - e_local])

            for n in range(nt):
                main_tile(n)
    _split_instruction_waits(nc, chain_sem, verbose=True)
    return nc


def build_mask(runs, nt=T // 512, n_lora=L, r=R):
    """mask[n, l*R + i, c] = 1 iff token n*512+c uses adapter l."""
    mask = np.zeros((nt, n_lora * r, 512), dtype=NP_BF16)
    for (a, b, li) in runs:
        for n in range(a // 512, (b + 511) // 512):
            c0, c1 = max(a, n * 512) - n * 512, min(b, (n + 1) * 512) - n * 512
            mask[n, li * r:(li + 1) * r, c0:c1] = 1.0
    return mask


def shard_inputs(x, weight, bias, lora_a, lora_b, runs):
    """Host-side shard + layout prep.  Returns the per-core input maps.

    x/W/A are cast to bf16; x/W are packed into [128, PK*512] blocks (PK
    k-tiles side by side, 4 KB per partition line).  Each core's blocks
    (and A's k-tiles) are rotated along the k-block axis so program block
    kb touches physical k-block (kb + core) % kb_cnt; block 0 is the
    core's own u shard.  Accumulation order changes per core, which is
    fine (float addition reordering within the psum group)."""
    x = np.asarray(x, dtype=np.float32)
    weight = np.asarray(weight, dtype=np.float32)
    bias = np.asarray(bias, dtype=np.float32)
    lora_a = np.asarray(lora_a, dtype=np.float32)
    lora_b = np.asarray(lora_b, dtype=np.float32)
    kt, nt, mt = H // 128, T // 512, O_S // 128
    kb_cnt = kt // PK

    # xB[kb, n, p, q*512 + c] = x[n*512 + c, (kb*PK + q)*128 + p]
    xB = np.ascontiguousarray(
        x.astype(NP_BF16)
        .reshape(nt, 512, kb_cnt, PK, 128)
        .transpose(2, 0, 4, 3, 1)
        .reshape(kb_cnt, nt, 128, PK * 512)
    )
    # aB[k, p, l*R + i] = lora_a[l, k*128 + p, i]
    aB = lora_a.astype(NP_BF16).transpose(1, 0, 2).reshape(kt, 128, L * R)
    mask = build_mask(runs)
    in_maps = []
    for c in range(N_CORES):
        sl = slice(c * O_S, (c + 1) * O_S)
        # wB[kb, p, q*O_S + o] = weight[sl][o, (kb*PK + q)*128 + p]
        wB = (
            weight[sl, :].T.astype(NP_BF16)
            .reshape(kb_cnt, PK, 128, O_S)
            .transpose(0, 2, 1, 3)
            .reshape(kb_cnt, 128, PK * O_S)
        )
        roll = (np.arange(kb_cnt) + c) % kb_cnt
        krot = (np.arange(kt) + c * (kt // N_CORES)) % kt
        # aT[p, j, :] = aB[krot[j], p, :]  (k on partitions, j-major free)
        aT_c = np.ascontiguousarray(aB[krot].transpose(1, 0, 2))
        ball_c = np.ascontiguousarray(
            lora_b[:, :, sl].reshape(L * R, O_S).astype(NP_BF16)
        )
        in_maps.append(
            {
                "xB": np.ascontiguousarray(xB[roll]),
                "wB": np.ascontiguousarray(wB[roll]),
                "aT": aT_c,
                "ball": ball_c,
                "umask": mask,
                "bias_r": np.ascontiguousarray(bias[sl].reshape(mt, 128).T),
            }
        )
    return in_maps


_program_cache: dict = {}
last_run_info: dict = {}


def kernel(x, weight, bias, lora_a, lora_b, indices, _trace=False):
    x = np.asarray(x)
    assert x.shape == (T, H), x.shape
    runs = runs_from_indices(np.asarray(indices), T)

    nc = _program_cache.get("prog")
    if nc is None:
        nc = build_program()
        _program_cache["prog"] = nc

    in_maps = shard_inputs(x, weight, bias, lora_a, lora_b, runs)

    if _trace:
        _install_ntff_shim()
    from concourse.bass_utils import run_bass_kernel_spmd

    res = run_bass_kernel_spmd(
        nc, in_maps, core_ids=list(range(N_CORES)), trace=_trace
    )
    last_run_info.clear()
    last_run_info.update(
        exec_time_ns=res.exec_time_ns,
        mean_exec_time_ns=getattr(res, "mean_exec_time_ns", None),
        instructions_and_trace=res.instructions_and_trace,
        profile_json=res.profile_json,
    )

    out = np.empty((T, O), dtype=np.float32)
    for c in range(N_CORES):
        out[:, c * O_S:(c + 1) * O_S] = res.results[c]["outT"].T
    return out


# revision 17
# speedup vs baseline: 1.0860x; 1.0860x over previous
"""ColumnParallelLinear + paged LoRA (SGMV) on 8 trn2 NeuronCores.

Math (per reference):
    out = x @ W^T + bias;  out[t] += x[t] @ A[l(t)] @ B[l(t)]
where l(t) is the adapter of token t's contiguous segment (from `indices`).

Sharding: column-parallel over the output dim.  Core c owns O/8 = 512
output columns: W shard, bias shard, B shard; x, A and the segment map are
replicated.

Precision: the whole compute path runs in bf16 operands with fp32 PSUM
accumulation (measured end-to-end relative error ~2e-3 against a 2e-2
budget).  bf16 halves the dominant DMA traffic (replicated x).

Device layout: everything is computed transposed (out^T [O_s, T]) so the
contraction dim H lands on SBUF partitions for both matmul operands.  x
and W are pre-packed on the host into [128, PK*512] blocks of PK=4
k-tiles so every DMA moves 4 KB per partition line.

LoRA handling (all matmuls full 128-wide contraction; 16-partition
matmuls measure ~2x slower per row on TRN2):
  *  u_all = x @ A_all  for ALL adapters at once ([L*R = 128] rank rows),
     masked per token down to the active adapter's 16 rows (mask built on
     the host from `indices`), then folded into the main GEMM as one
     extra contraction step: out += B_all^T @ u_masked.
  *  u_all is k-sharded across the cores (each core contracts its own H/8
     slice using x blocks it streams anyway; host-side k-rotation of the
     x/W blocks keeps the SPMD program identical on all cores) and ONE
     AllReduce assembles it.  The ncfw collective has ~50-90 us latency,
     so the first E tiles instead compute u_all locally (full k) as a 5th
     stationary column in their own j-loop; only tiles >= E use the
     collective result, by which time it has long completed.

The program is independent of `indices` (the segment map only enters
through the host-built mask tensor), so one compiled NEFF serves any
input.
"""

import numpy as np
import ml_dtypes

import concourse.bass as bass
import concourse.mybir as mybir
import concourse.tile as tile
from concourse.tile import TileContext
from concourse.vector_clock import ScopedClock

N_CORES = 8
T, H, O, R, L = 4096, 4096, 4096, 16, 8
O_S = O // N_CORES

F32 = mybir.dt.float32
F32R = mybir.dt.float32r
BF16 = mybir.dt.bfloat16
NP_BF16 = np.dtype(ml_dtypes.bfloat16)

PK = 4        # k-tiles packed per DMA block (8 KB partition lines in f32)
E_LOCAL = 3   # tiles computing u locally (covers AllReduce latency)

_drain_patched = False


def _patch_drain_waits():
    """walrus in this image rejects >1 sync-wait on the Tile exit Drain;
    spill the extra waits onto SP nops (semantically identical: SP
    executes them in order before the all-engine barrier)."""
    global _drain_patched
    if _drain_patched:
        return
    _drain_patched = True

    def _drain_and_barrier(self, tick_clock, wait_clock):
        drain_inst = self.nc.sync.drain()
        wait_clock.add_sem_waits(
            drain_inst.ins, ScopedClock({None: tick_clock.global_clock})
        )
        si = drain_inst.ins.sync_info
        if si is not None and si.on_wait and len(si.on_wait) > 1:
            waits = list(si.on_wait)
            si.on_wait = waits[:1]
            for w in waits[1:]:
                nop = self.nc.sync.nop()
                nop.ins.sync_info = mybir.SyncInfo(on_wait=[w], on_update=[])
        self.nc.all_engine_barrier()
        assert self.sems is not None
        popped = self.nc._tile_sem_poison_stack.pop()
        assert popped is self._sem_poison
        self.nc.clear_and_free_semaphores(list(self.sems.allocated().values()))
        self.nc.all_engine_barrier()

    TileContext._drain_and_barrier = _drain_and_barrier


def _split_instruction_waits(nc, chain_sem, max_waits=1, verbose=False):
    """walrus in this image encodes at most one sync-wait per instruction.

    Engine instructions execute in stream order, so extra waits can be
    peeled onto NoOps inserted immediately before the instruction.  For
    DMA transfers (whose single wait may be evaluated by the DGE queue
    rather than the issuing sequencer) all original waits are funnelled
    through SP NoOps that bump a dedicated chain semaphore; the DMA then
    waits for the chain count, which is equivalent to the conjunction of
    its original waits."""
    fn = nc.m.functions[0]
    stats = {}
    chain_used = False
    chain_count = 0
    for blk in fn.blocks:
        out = []
        changed = False
        for inst in blk.instructions:
            si = getattr(inst, "sync_info", None)
            if si is not None and si.on_wait and len(si.on_wait) > max_waits:
                stats[inst.opcode] = stats.get(inst.opcode, 0) + 1
                waits = list(si.on_wait)
                changed = True
                if "DMA" in inst.opcode:
                    chain_used = True
                    chain_count += 1
                    for idx, w in enumerate(waits):
                        nop = mybir.InstNoOp(
                            name=nc.get_next_instruction_name(),
                            engine=mybir.EngineType.SP,
                        )
                        upd = []
                        if idx == len(waits) - 1:
                            upd = [
                                mybir.SyncUpdate(
                                    sync_type="semaphore",
                                    id=chain_sem.num,
                                    update_mode="sem-inc",
                                    ant_name=chain_sem.name,
                                    update_value=1,
                                )
                            ]
                        nop.sync_info = mybir.SyncInfo(on_wait=[w], on_update=upd)
                        nc.register_instruction(nop)
                        out.append(nop)
                    si.on_wait = [
                        mybir.SyncWait(
                            sync_type="semaphore",
                            id=chain_sem.num,
                            wait_mode="sem-ge-imm",
                            ant_name=chain_sem.name,
                            wait_value=chain_count,
                        )
                    ]
                else:
                    for w in waits[:-max_waits]:
                        nop = mybir.InstNoOp(
                            name=nc.get_next_instruction_name(), engine=inst.engine
                        )
                        nop.sync_info = mybir.SyncInfo(on_wait=[w], on_update=[])
                        nc.register_instruction(nop)
                        out.append(nop)
                    si.on_wait = waits[-max_waits:]
            out.append(inst)
        if changed:
            blk.instructions = out
    if chain_used:
        # Reset the chain sem after the tail barrier so NEFF re-execution
        # starts from zero.
        nc.sync.sem_clear(chain_sem)
    if verbose and stats:
        print("split multi-wait instructions:", stats)
    return stats


def _install_ntff_shim():
    """Provide antenv.axon_hooks (absent in this image) so
    run_bass_kernel_spmd(trace=True) can capture NTFF profiles through
    the axon sidechannel, mirroring trn_boot's ctypes hook."""
    try:
        import antenv.axon_hooks  # noqa: F401
        return
    except ImportError:
        pass
    import contextlib
    import ctypes
    import sys
    import types

    import antenv

    mod = types.ModuleType("antenv.axon_hooks")
    holder = {}
    mod.set_axon_ntff_profile_hook = lambda h: holder.__setitem__("h", h)
    mod.get_axon_ntff_profile_hook = lambda: holder.get("h")
    sys.modules["antenv.axon_hooks"] = mod
    antenv.axon_hooks = mod

    so_path = "/opt/axon/libaxon_pjrt.so"
    lib = ctypes.CDLL(so_path)
    if not hasattr(lib, "axon_start_nrt_profile"):
        return
    lib.axon_start_nrt_profile.argtypes = [
        ctypes.POINTER(ctypes.c_int64),
        ctypes.c_size_t,
    ]
    lib.axon_start_nrt_profile.restype = ctypes.c_int64
    lib.axon_stop_nrt_profile.argtypes = [ctypes.c_char_p]
    lib.axon_stop_nrt_profile.restype = ctypes.c_int64

    @contextlib.contextmanager
    def _hook(output_dir, device_ids):
        import jax

        jax.devices()
        if device_ids:
            ids = (ctypes.c_int64 * len(device_ids))(*device_ids)
            rc = lib.axon_start_nrt_profile(ids, len(device_ids))
        else:
            rc = lib.axon_start_nrt_profile(None, 0)
        if rc != 0:
            raise RuntimeError(f"axon_start_nrt_profile rc={rc}")
        try:
            yield
        finally:
            n = lib.axon_stop_nrt_profile(str(output_dir).encode())
            print(f"ntff profile: {n} file(s) written to {output_dir}")

    mod.set_axon_ntff_profile_hook(_hook)


def runs_from_indices(indices: np.ndarray, n_tokens: int) -> tuple:
    """Expand `indices` into maximal contiguous token runs with a fixed
    adapter, mirroring the reference searchsorted semantics exactly
    (including the negative-index wrap for tokens before starts[0])."""
    starts = np.asarray(indices[:-1, 0], dtype=np.int64)
    seg_lora = np.asarray(indices[:-1, 1], dtype=np.int64)
    tok = np.arange(n_tokens, dtype=np.int64)
    seg = np.searchsorted(starts, tok, side="right") - 1
    tok_lora = seg_lora[seg]  # seg == -1 wraps to the last segment, like jnp
    change = np.flatnonzero(np.diff(tok_lora)) + 1
    run_starts = np.concatenate(([0], change))
    run_ends = np.concatenate((change, [n_tokens]))
    return tuple(
        (int(a), int(b), int(tok_lora[a])) for a, b in zip(run_starts, run_ends)
    )


def build_program(t=T, h=H, o_s=O_S, r=R, n_lora=L, n_shards=N_CORES,
                  e_local=E_LOCAL):
    """Emit the single-core Tile program (SPMD across the cores)."""
    _patch_drain_waits()
    assert t % 512 == 0 and h % 128 == 0 and o_s % 128 == 0
    kt = h // 128          # contraction tiles
    nt = t // 512          # token (moving) tiles
    mt = o_s // 128        # output-partition tiles
    ra = n_lora * r        # all-adapter rank width
    assert ra == 128
    assert kt % n_shards == 0
    ksh = kt // n_shards   # k-tiles per core's u shard
    assert ksh == PK, "prepass consumes exactly the first packed block"
    kb_cnt = kt // PK
    e_local = max(1, min(e_local, nt))

    nc = bass.Bass("TRN2", num_devices=n_shards)
    chain_sem = nc.alloc_semaphore("dma_wait_chain")
    xB_d = nc.dram_tensor("xB", [kb_cnt, nt, 128, PK * 512], F32R,
                          kind="ExternalInput")
    wB_d = nc.dram_tensor("wB", [kb_cnt, 128, PK * o_s], F32R,
                          kind="ExternalInput")
    aT_d = nc.dram_tensor("aT", [128, kt * ra // 128, 128], F32R,
                          kind="ExternalInput")
    ball_d = nc.dram_tensor("ball", [128, o_s], F32R, kind="ExternalInput")
    mask_d = nc.dram_tensor("umask", [nt, 128, 512], BF16, kind="ExternalInput")
    bias_d = nc.dram_tensor("bias_r", [128, mt], F32, kind="ExternalInput")
    out_d = nc.dram_tensor("outT", [o_s, t], F32, kind="ExternalOutput")

    aT_v = aT_d[:].rearrange("p a b -> p (a b)")

    with TileContext(nc) as tc:
        with (
            tc.tile_pool(name="resident", bufs=1) as res,
            tc.tile_pool(name="xs", bufs=5) as xs,
            tc.tile_pool(name="xpre", bufs=max(1, nt - e_local)) as xpre,
            tc.tile_pool(name="masks", bufs=3) as maskp,
            tc.tile_pool(name="us", bufs=4) as us,
            tc.tile_pool(name="ufs", bufs=max(1, nt - e_local)) as ufs,
            tc.tile_pool(name="outs", bufs=6) as outs,
            tc.tile_pool(name="dramp", bufs=1, space="DRAM") as dramp,
            tc.tile_pool(name="psum_o", bufs=6, space="PSUM") as psum_o,
            tc.tile_pool(name="psum_u", bufs=2, space="PSUM") as psum_u,
        ):
            w_sb = res.tile([128, kt * o_s], F32R, tag="w", name="w_sb")
            a_sb = res.tile([128, kt * ra], F32R, tag="a", name="a_sb")
            ball_sb = res.tile([128, o_s], F32R, tag="ball", name="ball_sb")
            bias_sb = res.tile([128, mt], F32, tag="bias", name="bias_sb")

            up_t = dramp.tile([nt, 128, 512], F32, tag="upart", name="up_t")
            uf_t = dramp.tile(
                [max(1, nt - e_local), 128, 512], F32, tag="ufull",
                name="uf_t", addr_space="Shared",
            )

            xpre_tiles = {}
            mask_tiles = {}
            uf_tiles = {}

            def load_w(kb):
                nc.sync.dma_start(
                    w_sb[:, kb * PK * o_s:(kb + 1) * PK * o_s], wB_d[kb]
                )

            def load_xpre(n):
                xt = xpre.tile([128, PK * 512], F32R, tag="xp", name="xp")
                xpre_tiles[n] = xt
                nc.sync.dma_start(xt[:], xB_d[0, n])

            def prepass(n):
                pu = psum_u.tile([128, 512], F32, tag="pu", name="pu")
                xt = xpre_tiles[n]
                for jj in range(ksh):
                    nc.tensor.matmul(
                        pu[:],
                        a_sb[:, jj * ra:(jj + 1) * ra],
                        xt[:, jj * 512:(jj + 1) * 512],
                        start=(jj == 0),
                        stop=(jj == ksh - 1),
                    )
                up_sb = us.tile([128, 512], F32, tag="up", name="up_sb")
                nc.vector.tensor_copy(up_sb[:], pu[:])
                nc.sync.dma_start(up_t[n], up_sb[:])

            # ---- warm-up: only what main tile 0 needs right away.  The
            # first tiles are DMA-paced behind the one-time W/A freight, so
            # everything else (prepass x blocks, the AllReduce, masks) is
            # threaded into the first two tiles' k-loops instead of being
            # front-loaded.  Tile 0's j<ksh steps use the core's own
            # A-shard, which is the first slice of the rotated a_sb.
            nc.sync.dma_start(a_sb[:, :ksh * ra], aT_v[:, :ksh * ra])
            load_w(0)
            load_w(1)
            nc.sync.dma_start(a_sb[:, ksh * ra:], aT_v[:, ksh * ra:])
            nc.sync.dma_start(ball_sb[:], ball_d[:])
            nc.sync.dma_start(bias_sb[:], bias_d[:])

            def main_tile(n):
                c0 = n * 512
                local = n < e_local
                mk = maskp.tile([128, 512], BF16, tag="mk", name="mk")
                mask_tiles[n] = mk
                nc.sync.dma_start(mk[:], mask_d[n])
                ptiles = [
                    psum_o.tile([128, 512], F32, tag="po", name="po")
                    for _ in range(mt)
                ]
                if local:
                    pu = psum_u.tile([128, 512], F32, tag="pu", name="pu")
                for kb in range(kb_cnt):
                    if n == 0 and kb + 2 < kb_cnt:
                        # stream the rest of W just ahead of its first use
                        load_w(kb + 2)
                    if n == 1 and 1 <= kb and e_local + kb - 1 < nt:
                        # prepass x blocks (= later tiles' kb=0 blocks)
                        load_xpre(e_local + kb - 1)
                    if kb == 0 and n in xpre_tiles:
                        xt = xpre_tiles[n]
                    else:
                        xt = xs.tile([128, PK * 512], F32R, tag="x", name="xt")
                        nc.sync.dma_start(xt[:], xB_d[kb, n])
                    for q in range(PK):
                        j = kb * PK + q
                        mv = xt[:, q * 512:(q + 1) * 512]
                        for m in range(mt):
                            nc.tensor.matmul(
                                ptiles[m][:],
                                w_sb[:, j * o_s + m * 128:j * o_s + (m + 1) * 128],
                                mv,
                                start=(j == 0),
                                stop=False,
                            )
                        if local:
                            nc.tensor.matmul(
                                pu[:],
                                a_sb[:, j * ra:(j + 1) * ra],
                                mv,
                                start=(j == 0),
                                stop=(j == kt - 1),
                            )
                if n == e_local and e_local < nt:
                    # The AllReduce has had ~2.5 tiles of runway; pull its
                    # results into SBUF in one batch (these DMAs wait on
                    # the collective, so they are kept off the x-stream's
                    # critical path until now).
                    for nn in range(e_local, nt):
                        uf = ufs.tile([128, 512], F32, tag="uf", name="uf")
                        uf_tiles[nn] = uf
                        nc.sync.dma_start(uf[:], uf_t[nn - e_local])
                # masked u for this tile, then fold B_all in as one extra
                # contraction step per output block (f32r via bitcast).
                um = us.tile([128, 512], F32R, tag="um", name="um")
                if local:
                    nc.vector.tensor_tensor(
                        um[:], pu[:], mask_tiles[n][:], mybir.AluOpType.mult
                    )
                else:
                    nc.vector.tensor_tensor(
                        um[:], uf_tiles[n][:], mask_tiles[n][:],
                        mybir.AluOpType.mult,
                    )
                for m in range(mt):
                    nc.tensor.matmul(
                        ptiles[m][:],
                        ball_sb[:, m * 128:(m + 1) * 128],
                        um[:],
                        start=False,
                        stop=True,
                    )
                for m in range(mt):
                    ot = outs.tile([128, 512], F32, tag="o", name="ot")
                    nc.vector.tensor_scalar_add(
                        ot[:], ptiles[m][:], bias_sb[:, m:m + 1]
                    )
                    nc.sync.dma_start(out_d[m * 128:(m + 1) * 128, c0:c0 + 512],
                                      ot[:])

            for n in range(nt):
                main_tile(n)
                if n == 1 and e_local < nt:
                    # prepasses + the single AllReduce, issued while tiles
                    # 2..e_local-1 still have locally-computed u; the ncfw
                    # collective needs ~85 us from issue to completion.
                    for pn in range(e_local, nt):
                        prepass(pn)
                    nc.gpsimd.collective_compute(
                        "AllReduce",
                        mybir.AluOpType.add,
                        replica_groups=[list(range(n_shards))],
                        ins=[up_t[e_local:nt]],
                        outs=[uf_t[:]],
                    )
    _split_instruction_waits(nc, chain_sem, verbose=True)
    return nc


def build_mask(runs, nt=T // 512, n_lora=L, r=R):
    """mask[n, l*R + i, c] = 1 iff token n*512+c uses adapter l."""
    mask = np.zeros((nt, n_lora * r, 512), dtype=NP_BF16)
    for (a, b, li) in runs:
        for n in range(a // 512, (b + 511) // 512):
            c0, c1 = max(a, n * 512) - n * 512, min(b, (n + 1) * 512) - n * 512
            mask[n, li * r:(li + 1) * r, c0:c1] = 1.0
    return mask


def shard_inputs(x, weight, bias, lora_a, lora_b, runs):
    """Host-side shard + layout prep.  Returns the per-core input maps.

    x/W/A are cast to bf16; x/W are packed into [128, PK*512] blocks (PK
    k-tiles side by side, 4 KB per partition line).  Each core's blocks
    (and A's k-tiles) are rotated along the k-block axis so program block
    kb touches physical k-block (kb + core) % kb_cnt; block 0 is the
    core's own u shard.  Accumulation order changes per core, which is
    fine (float addition reordering within the psum group)."""
    x = np.asarray(x, dtype=np.float32)
    weight = np.asarray(weight, dtype=np.float32)
    bias = np.asarray(bias, dtype=np.float32)
    lora_a = np.asarray(lora_a, dtype=np.float32)
    lora_b = np.asarray(lora_b, dtype=np.float32)
    kt, nt, mt = H // 128, T // 512, O_S // 128
    kb_cnt = kt // PK

    # xB[kb, n, p, q*512 + c] = x[n*512 + c, (kb*PK + q)*128 + p]
    xB = np.ascontiguousarray(
        x.reshape(nt, 512, kb_cnt, PK, 128)
        .transpose(2, 0, 4, 3, 1)
        .reshape(kb_cnt, nt, 128, PK * 512)
    )
    # aB[k, p, l*R + i] = lora_a[l, k*128 + p, i]
    aB = lora_a.transpose(1, 0, 2).reshape(kt, 128, L * R)
    mask = build_mask(runs)
    in_maps = []
    for c in range(N_CORES):
        sl = slice(c * O_S, (c + 1) * O_S)
        # wB[kb, p, q*O_S + o] = weight[sl][o, (kb*PK + q)*128 + p]
        wB = (
            weight[sl, :].T
            .reshape(kb_cnt, PK, 128, O_S)
            .transpose(0, 2, 1, 3)
            .reshape(kb_cnt, 128, PK * O_S)
        )
        roll = (np.arange(kb_cnt) + c) % kb_cnt
        krot = (np.arange(kt) + c * (kt // N_CORES)) % kt
        # aT[p, j, :] = aB[krot[j], p, :]  (k on partitions, j-major free)
        aT_c = np.ascontiguousarray(aB[krot].transpose(1, 0, 2))
        ball_c = np.ascontiguousarray(lora_b[:, :, sl].reshape(L * R, O_S))
        in_maps.append(
            {
                "xB": np.ascontiguousarray(xB[roll]),
                "wB": np.ascontiguousarray(wB[roll]),
                "aT": aT_c,
                "ball": ball_c,
                "umask": mask,
                "bias_r": np.ascontiguousarray(bias[sl].reshape(mt, 128).T),
            }
        )
    return in_maps


_program_cache: dict = {}
last_run_info: dict = {}


def kernel(x, weight, bias, lora_a, lora_b, indices, _trace=False):
    x = np.asarray(x)
    assert x.shape == (T, H), x.shape
    runs = runs_from_indices(np.asarray(indices), T)

    nc = _program_cache.get("prog")
    if nc is None:
        nc = build_program()
        _program_cache["prog"] = nc

    in_maps = shard_inputs(x, weight, bias, lora_a, lora_b, runs)

    if _trace:
        _install_ntff_shim()
    from concourse.bass_utils import run_bass_kernel_spmd

    res = run_bass_kernel_spmd(
        nc, in_maps, core_ids=list(range(N_CORES)), trace=_trace
    )
    last_run_info.clear()
    last_run_info.update(
        exec_time_ns=res.exec_time_ns,
        mean_exec_time_ns=getattr(res, "mean_exec_time_ns", None),
        instructions_and_trace=res.instructions_and_trace,
        profile_json=res.profile_json,
    )

    out = np.empty((T, O), dtype=np.float32)
    for c in range(N_CORES):
        out[:, c * O_S:(c + 1) * O_S] = res.results[c]["outT"].T
    return out
